# revision 9
# baseline (speedup 1.0000x reference)
"""Trainium2 Bass kernel for nn_Attention_81037442941065.

Dual-attention module (spatial [b,h,n,n] + channel [b,h,d,d]) with
B=2, N=2048, DIM=1024, 16 heads of d=64.

Sharding: 8 cores = (2 batches) x (4 head-groups of 4 heads).
Each core computes its batch/head-group slice end-to-end and produces a
partial (over head groups) output projection; the host sums the 4 group
partials per batch (the "all-reduce after to_out") and adds b_out.

Per-core layouts (everything "T" is [channels, tokens]):
  z1T, yhT   : [256, 2048]  (transposed projections, head h at rows 64*(h%2)
                             of tile h//2)
  xh_aug     : 16 tiles [128, 260] (natural layout per 128-token chunk;
               per head 65 cols = 64 channels + a ones column so the AV
               matmul also produces the softmax denominators)
  spatial    : S^T = yh @ z1^T computed [keys, queries]; exp on ScalarE
               (scale 1/8 fused, no max subtraction - logits are small);
               AV matmul lhsT=[xh|1] accumulates over key chunks -> rows
               0..63 = unnormalized out1^T, row 64 = sum of exp.
  channel    : [64,64] per head, one PSUM bank each; softmax via
               Exp+accum_out and per-partition reciprocal multiply.
All matmul inputs live in float32r (1 cycle/row on TRN2 for moving dim
>= 256, vs 4 cycles/row for plain float32); walrus requires fp32r
matmul operands to be produced as fp32r, so every SBUF tile that feeds
the PE is allocated float32r and the producing engine op rounds into it.
"""

import sys

for _p in ("/opt/trn_rl_repo", "/opt/pypackages"):
    if _p not in sys.path:
        sys.path.insert(0, _p)

import numpy as np
from contextlib import ExitStack

import concourse.bacc as bacc
import concourse.mybir as mybir
import concourse.tile as tile
from concourse.bass_utils import run_bass_kernel_spmd

F32 = mybir.dt.float32
F32R = mybir.dt.float32r
EXP = mybir.ActivationFunctionType.Exp

B, N, DIM = 2, 2048, 1024
HEADS, DH = 16, 64
G = 4              # head groups == cores per batch
HG = HEADS // G    # heads per group (4)
CIN = HG * DH      # inner channels per core (256)
NCORES = 8
KC = DIM // 128    # contraction chunks for projections (8)
NCH = N // 128     # 128-token chunks (16)
SCALE = DH ** -0.5            # 1/8
CM_SCALE = SCALE / (N / DH)   # 1/256


def _build_program():
    nc = bacc.Bacc(
        "TRN2", target_bir_lowering=False, debug=False, num_devices=NCORES
    )

    # ---- DRAM I/O ----
    xT_d = nc.dram_tensor("xT", [DIM, N], F32R, kind="ExternalInput").ap()
    yT_d = nc.dram_tensor("yT", [DIM, N], F32R, kind="ExternalInput").ap()
    zT_d = nc.dram_tensor("zT", [DIM, N], F32R, kind="ExternalInput").ap()
    wsa1_d = nc.dram_tensor("w_sa1", [DIM, CIN], F32R, kind="ExternalInput").ap()
    wsa2_d = nc.dram_tensor("w_sa2", [DIM, CIN], F32R, kind="ExternalInput").ap()
    wse1_d = nc.dram_tensor("w_se1", [DIM, CIN], F32R, kind="ExternalInput").ap()
    wse2_d = nc.dram_tensor("w_se2", [DIM, CIN], F32R, kind="ExternalInput").ap()
    wout_d = nc.dram_tensor("w_out", [CIN, DIM], F32R, kind="ExternalInput").ap()
    outT_d = nc.dram_tensor("outT", [DIM, N], F32, kind="ExternalOutput").ap()

    with tile.TileContext(nc) as tc, ExitStack() as ctx:
        ppool = ctx.enter_context(tc.tile_pool(name="persist", bufs=1))
        cpool = ctx.enter_context(tc.tile_pool(name="cat", bufs=1))

        # Persistent projection outputs.
        z1T = [ppool.tile([128, N], F32R, tag=f"z1T{m}", name=f"z1T{m}")
               for m in range(2)]
        yhT = [ppool.tile([128, N], F32R, tag=f"yhT{m}", name=f"yhT{m}")
               for m in range(2)]
        xh_aug = [ppool.tile([128, HG * (DH + 1)], F32R, tag=f"xa{i}",
                             name=f"xa{i}") for i in range(NCH)]
        secm_sb = [ppool.tile([128, DH], F32R, tag=f"cm{p}", name=f"cm{p}")
                   for p in range(2)]
        rs = [ppool.tile([64, 1], F32, tag=f"rs{h}", name=f"rs{h}")
              for h in range(HG)]
        rcm = [ppool.tile([64, 1], F32, tag=f"rcm{h}", name=f"rcm{h}")
               for h in range(HG)]

        # cat^T staging: one [64, N] tile per head (head h covers this
        # core's inner channels [64h, 64h+64)); the final projection
        # contracts them with matching 64-row slices of w_out.
        cat4 = [cpool.tile([64, N], F32R, tag=f"cat{h}", name=f"cat{h}")
                for h in range(HG)]

        # ---------------- Phase X: xh (natural, augmented) ----------------
        with tc.tile_pool(name="xin", bufs=1) as xpool, \
             tc.tile_pool(name="psx", bufs=4, space="PSUM") as psx:
            wse1_t = [xpool.tile([128, CIN], F32R, tag=f"wse1_{k}",
                                 name=f"wse1_{k}") for k in range(KC)]
            for k in range(KC):
                nc.sync.dma_start(wse1_t[k][:], wse1_d[k * 128:(k + 1) * 128, :])
            xTt = [xpool.tile([128, N], F32R, tag=f"x{k}", name=f"x{k}")
                   for k in range(KC)]
            for k in range(KC):
                nc.sync.dma_start(xTt[k][:], xT_d[k * 128:(k + 1) * 128, :])

            for i in range(NCH):
                ps = psx.tile([128, CIN], F32, tag="pj", name=f"psx{i}")
                for k in range(KC):
                    nc.tensor.matmul(
                        ps[:],
                        lhsT=xTt[k][:, i * 128:(i + 1) * 128],
                        rhs=wse1_t[k][:],
                        start=(k == 0), stop=(k == KC - 1),
                    )
                # strided copy of the 4 head blocks + ones columns
                # (memset can't write fp32r; ACT Copy with scale=0 bias=1
                # produces rounded 1.0s)
                src = ps[:].rearrange("p (h c) -> p h c", c=DH)
                dst = xh_aug[i][:].rearrange("p (h c) -> p h c", c=DH + 1)
                nc.vector.tensor_copy(dst[:, :, 0:DH], src)
                nc.scalar.activation(dst[:, :, DH:DH + 1], src[:, :, 0:1],
                                     mybir.ActivationFunctionType.Copy,
                                     bias=1.0, scale=0.0)

        # ---------------- Phase Z: z1T + channel-attn logits ----------------
        with tc.tile_pool(name="zin", bufs=1) as zpool, \
             tc.tile_pool(name="psz", bufs=4, space="PSUM") as psz, \
             tc.tile_pool(name="pscm", bufs=1, space="PSUM") as pscm:
            wsa1_t = [zpool.tile([128, CIN], F32R, tag=f"wsa1_{k}",
                                 name=f"wsa1_{k}") for k in range(KC)]
            wse2_t = [zpool.tile([128, CIN], F32R, tag=f"wse2_{k}",
                                 name=f"wse2_{k}") for k in range(KC)]
            for k in range(KC):
                nc.sync.dma_start(wsa1_t[k][:], wsa1_d[k * 128:(k + 1) * 128, :])
                nc.sync.dma_start(wse2_t[k][:], wse2_d[k * 128:(k + 1) * 128, :])
            zTt = [zpool.tile([128, N], F32R, tag=f"z{k}", name=f"z{k}")
                   for k in range(KC)]
            for k in range(KC):
                nc.sync.dma_start(zTt[k][:], zT_d[k * 128:(k + 1) * 128, :])

            # one PSUM bank per head (separate accumulation groups may not
            # share a 2KB zero region); fp32r matmul outputs must start at
            # partition 0, so heads are packed into pairs only afterwards
            cmps = [pscm.tile([64, DH], F32, tag=f"cmp{h}", name=f"cmp{h}")
                    for h in range(HG)]

            # z1T (transposed projection)
            for m in range(2):
                for nb in range(4):
                    ps = psz.tile([128, 512], F32, tag="pj", name=f"psz{m}{nb}")
                    for k in range(KC):
                        nc.tensor.matmul(
                            ps[:],
                            lhsT=wsa1_t[k][:, m * 128:(m + 1) * 128],
                            rhs=zTt[k][:, nb * 512:(nb + 1) * 512],
                            start=(k == 0), stop=(k == KC - 1),
                        )
                    nc.any.tensor_copy(z1T[m][:, nb * 512:(nb + 1) * 512], ps[:])

            # z2 (natural, streamed) + channel-attn logit accumulation
            for i in range(NCH):
                ps2 = psz.tile([128, 512], F32, tag="pj", name=f"psz2_{i}")
                for k in range(KC):
                    nc.tensor.matmul(
                        ps2[:, 0:CIN],
                        lhsT=zTt[k][:, i * 128:(i + 1) * 128],
                        rhs=wse2_t[k][:],
                        start=(k == 0), stop=(k == KC - 1),
                    )
                z2n = zpool.tile([128, CIN], F32R, tag="z2n", bufs=3,
                                 name=f"z2n{i}")
                nc.any.tensor_copy(z2n[:], ps2[:, 0:CIN])
                for h in range(HG):
                    nc.tensor.matmul(
                        cmps[h][:],
                        lhsT=xh_aug[i][:, 65 * h:65 * h + DH],
                        rhs=z2n[:, DH * h:DH * (h + 1)],
                        start=(i == 0), stop=(i == NCH - 1),
                    )

            # channel-attn softmax per head at partitions 0-63, then DMA
            # into the pair-packed secm_sb position (engines cannot shift
            # partitions; DMA can)
            for h in range(HG):
                p_, off = h // 2, 64 * (h % 2)
                st = zpool.tile([64, DH], F32R, tag="cmstage", bufs=4,
                                name=f"cmstage{h}")
                nc.scalar.activation(st[:], cmps[h][:], EXP,
                                     scale=CM_SCALE,
                                     accum_out=rs[h][0:64, 0:1])
                nc.vector.reciprocal(rcm[h][0:64, 0:1], rs[h][0:64, 0:1])
                nc.vector.tensor_scalar_mul(st[:], st[:], rcm[h][0:64, 0:1])
                nc.sync.dma_start(secm_sb[p_][off:off + 64, :], st[:])

        # ---------------- Phase Y: yhT + out2 ----------------
        with tc.tile_pool(name="yin", bufs=1) as ypool, \
             tc.tile_pool(name="psy", bufs=4, space="PSUM") as psy:
            wsa2_t = [ypool.tile([128, CIN], F32R, tag=f"wsa2_{k}",
                                 name=f"wsa2_{k}") for k in range(KC)]
            for k in range(KC):
                nc.sync.dma_start(wsa2_t[k][:], wsa2_d[k * 128:(k + 1) * 128, :])
            yTt = [ypool.tile([128, N], F32R, tag=f"y{k}", name=f"y{k}")
                   for k in range(KC)]
            for k in range(KC):
                nc.sync.dma_start(yTt[k][:], yT_d[k * 128:(k + 1) * 128, :])

            for m in range(2):
                for nb in range(4):
                    ps = psy.tile([128, 512], F32, tag="pj", name=f"psy{m}{nb}")
                    for k in range(KC):
                        nc.tensor.matmul(
                            ps[:],
                            lhsT=wsa2_t[k][:, m * 128:(m + 1) * 128],
                            rhs=yTt[k][:, nb * 512:(nb + 1) * 512],
                            start=(k == 0), stop=(k == KC - 1),
                        )
                    nc.any.tensor_copy(yhT[m][:, nb * 512:(nb + 1) * 512], ps[:])

            # out2^T = secm^T @ yh^T, written into cat staging (out1 is
            # added on top later during the spatial phase tails)
            for h in range(HG):
                p_, off = h // 2, 64 * (h % 2)
                for nb in range(4):
                    pso = psy.tile([128, 512], F32, tag="pj", name=f"pso{h}{nb}")
                    nc.tensor.matmul(
                        pso[0:64, :],
                        lhsT=secm_sb[p_][off:off + 64, :],
                        rhs=yhT[p_][off:off + 64, nb * 512:(nb + 1) * 512],
                        start=True, stop=True,
                    )
                    nc.any.tensor_copy(cat4[h][:, nb * 512:(nb + 1) * 512],
                                       pso[0:64, :])

        # -------- Spatial attention + output projection --------
        with tc.tile_pool(name="spat", bufs=1) as spool, \
             tc.tile_pool(name="pt", bufs=4) as ptpool, \
             tc.tile_pool(name="tails", bufs=3) as tpool, \
             tc.tile_pool(name="oout", bufs=4) as opool, \
             tc.tile_pool(name="psS", bufs=2, space="PSUM") as psS, \
             tc.tile_pool(name="psAV", bufs=4, space="PSUM") as psAV:

            # w_out as four 64-row slices (base partition 0) matching cat4
            wq = [spool.tile([64, DIM], F32R, tag=f"wq{q}", name=f"wq{q}")
                  for q in range(HG)]
            for q in range(HG):
                nc.sync.dma_start(wq[q][:], wout_d[q * 64:(q + 1) * 64, :])

            for p_ in range(2):
                for ib in range(2):  # query mega-block of 1024
                    avs = [psAV.tile([128, 512], F32, tag="av",
                                     name=f"av{p_}{ib}{q}") for q in range(4)]
                    for j in range(NCH):  # key chunks
                        for hh in range(2):
                            h = 2 * p_ + hh
                            off = 64 * hh
                            spt = psS.tile([128, 1024], F32, tag="S",
                                           name=f"S{p_}{ib}{j}{hh}")
                            for s in range(2):
                                icol = ib * 1024 + s * 512
                                nc.tensor.matmul(
                                    spt[:, s * 512:(s + 1) * 512],
                                    lhsT=yhT[p_][off:off + 64,
                                                 j * 128:(j + 1) * 128],
                                    rhs=z1T[p_][off:off + 64, icol:icol + 512],
                                    start=True, stop=True,
                                )
                            ptt = ptpool.tile([128, 1024], F32R, tag="pt",
                                              name=f"pt{p_}{ib}{j}{hh}")
                            nc.scalar.activation(ptt[:], spt[:], EXP,
                                                 scale=SCALE)
                            for s in range(2):
                                nc.tensor.matmul(
                                    avs[2 * hh + s][0:DH + 1, :],
                                    lhsT=xh_aug[j][:, 65 * h:65 * h + DH + 1],
                                    rhs=ptt[:, s * 512:(s + 1) * 512],
                                    start=(j == 0), stop=(j == NCH - 1),
                                )
                    # tails: normalize out1 and add into cat staging
                    for hh in range(2):
                        h = 2 * p_ + hh
                        for s in range(2):
                            av = avs[2 * hh + s]
                            icol = ib * 1024 + s * 512
                            rc = tpool.tile([1, 512], F32, tag="rc",
                                            name=f"rc{p_}{ib}{hh}{s}")
                            nc.vector.reciprocal(rc[:], av[DH:DH + 1, :])
                            bc = tpool.tile([64, 512], F32, tag="bc",
                                            name=f"bc{p_}{ib}{hh}{s}")
                            nc.gpsimd.partition_broadcast(bc[:], rc[:])
                            tmp = tpool.tile([64, 512], F32, tag="tmp",
                                             name=f"tmp{p_}{ib}{hh}{s}")
                            nc.vector.tensor_mul(tmp[:], av[0:DH, :], bc[:])
                            dst = cat4[h][:, icol:icol + 512]
                            nc.vector.tensor_add(dst, tmp[:], dst)

            # final projection: out^T = w_out_g^T @ cat^T (partial over
            # groups), contracting the four 64-channel chunks
            for d in range(8):
                for nb in range(4):
                    psf = psS.tile([128, 512], F32, tag="S", name=f"psf{d}{nb}")
                    for q in range(HG):
                        nc.tensor.matmul(
                            psf[:],
                            lhsT=wq[q][:, d * 128:(d + 1) * 128],
                            rhs=cat4[q][:, nb * 512:(nb + 1) * 512],
                            start=(q == 0), stop=(q == HG - 1),
                        )
                    ob = opool.tile([128, 512], F32, tag="ob", name=f"ob{d}{nb}")
                    nc.any.tensor_copy(ob[:], psf[:])
                    nc.sync.dma_start(
                        outT_d[d * 128:(d + 1) * 128, nb * 512:(nb + 1) * 512],
                        ob[:],
                    )

    nc.compile()
    return nc


_NC_CACHE = {}


def _get_program():
    if "nc" not in _NC_CACHE:
        _NC_CACHE["nc"] = _build_program()
    return _NC_CACHE["nc"]


def _prep_input_maps(x, y, z, w_sa1, w_sa2, w_se1, w_se2, w_out):
    f32 = lambda a: np.ascontiguousarray(np.asarray(a, dtype=np.float32))
    maps = []
    for c in range(NCORES):
        b, g = divmod(c, G)
        sl = slice(g * CIN, (g + 1) * CIN)
        maps.append({
            "xT": f32(np.asarray(x)[b].T),
            "yT": f32(np.asarray(y)[b].T),
            "zT": f32(np.asarray(z)[b].T),
            "w_sa1": f32(np.asarray(w_sa1)[:, sl]),
            "w_sa2": f32(np.asarray(w_sa2)[:, sl]),
            "w_se1": f32(np.asarray(w_se1)[:, sl]),
            "w_se2": f32(np.asarray(w_se2)[:, sl]),
            "w_out": f32(np.asarray(w_out)[sl, :]),
        })
    return maps


def run(inputs, trace=False, trace_kwargs=None):
    """Run on hardware; returns (full_output, BassKernelResults)."""
    nc = _get_program()
    in_maps = _prep_input_maps(
        inputs["x"], inputs["y"], inputs["z"],
        inputs["w_sa1"], inputs["w_sa2"], inputs["w_se1"], inputs["w_se2"],
        inputs["w_out"],
    )
    res = run_bass_kernel_spmd(
        nc, in_maps, list(range(NCORES)), trace=trace,
        trace_kwargs=trace_kwargs or {},
    )
    out = np.zeros((B, N, DIM), dtype=np.float32)
    for c in range(NCORES):
        b, _g = divmod(c, G)
        out[b] += res.results[c]["outT"].T
    out += np.asarray(inputs["b_out"], dtype=np.float32)
    return out, res


def kernel(**inputs) -> np.ndarray:
    out, _ = run(inputs, trace=False)
    return out


# revision 10
# speedup vs baseline: 1.0862x; 1.0862x over previous
"""Trainium2 Bass kernel for nn_Attention_81037442941065.

Dual-attention module (spatial [b,h,n,n] + channel [b,h,d,d]) with
B=2, N=2048, DIM=1024, 16 heads of d=64.

Sharding: 8 cores = (2 batches) x (4 head-groups of 4 heads).
Each core computes its batch/head-group slice end-to-end and produces a
partial (over head groups) output projection; the host sums the 4 group
partials per batch (the "all-reduce after to_out") and adds b_out.

Dtypes: inputs x/y/z and the four projection weights are cast to bf16 on
the host (halves DMA + SBUF so all inputs stay resident; enables fast
weight load). Everything downstream of the projections runs in float32r
(1 cycle/row on the PE like bf16, ~11-bit mantissa) with fp32 PSUM.

Per-core layouts (everything "T" is [channels, tokens]):
  z1T, yhT   : [256, 2048]  (transposed projections, head h at rows 64*(h%2)
                             of tile h//2)
  xh_aug     : 16 tiles [128, 260] (natural layout per 128-token chunk;
               per head 65 cols = 64 channels + a ones column so the AV
               matmul also produces the softmax denominators)
  spatial    : S^T = yh @ z1^T computed [keys, queries]; exp on ScalarE
               (scale 1/8 fused, no max subtraction - logits are small);
               AV matmul lhsT=[xh|1] accumulates over key chunks -> rows
               0..63 = unnormalized out1^T, row 64 = sum of exp.
  channel    : [64,64] per head, one PSUM bank each; softmax via
               Exp+accum_out and per-partition reciprocal multiply.
"""

import sys

for _p in ("/opt/trn_rl_repo", "/opt/pypackages"):
    if _p not in sys.path:
        sys.path.insert(0, _p)

import ml_dtypes
import numpy as np
from contextlib import ExitStack

import concourse.bacc as bacc
import concourse.mybir as mybir
import concourse.tile as tile
from concourse.bass_utils import run_bass_kernel_spmd

F32 = mybir.dt.float32
F32R = mybir.dt.float32r
BF16 = mybir.dt.bfloat16
EXP = mybir.ActivationFunctionType.Exp

B, N, DIM = 2, 2048, 1024
HEADS, DH = 16, 64
G = 4              # head groups == cores per batch
HG = HEADS // G    # heads per group (4)
CIN = HG * DH      # inner channels per core (256)
NCORES = 8
KC = DIM // 128    # contraction chunks for projections (8)
NCH = N // 128     # 128-token chunks (16)
SCALE = DH ** -0.5            # 1/8
CM_SCALE = SCALE / (N / DH)   # 1/256


def _build_program():
    nc = bacc.Bacc(
        "TRN2", target_bir_lowering=False, debug=False, num_devices=NCORES
    )

    # ---- DRAM I/O ----
    xT_d = nc.dram_tensor("xT", [DIM, N], BF16, kind="ExternalInput").ap()
    yT_d = nc.dram_tensor("yT", [DIM, N], BF16, kind="ExternalInput").ap()
    zT_d = nc.dram_tensor("zT", [DIM, N], BF16, kind="ExternalInput").ap()
    wsa1_d = nc.dram_tensor("w_sa1", [DIM, CIN], BF16, kind="ExternalInput").ap()
    wsa2_d = nc.dram_tensor("w_sa2", [DIM, CIN], BF16, kind="ExternalInput").ap()
    wse1_d = nc.dram_tensor("w_se1", [DIM, CIN], BF16, kind="ExternalInput").ap()
    wse2_d = nc.dram_tensor("w_se2", [DIM, CIN], BF16, kind="ExternalInput").ap()
    wout_d = nc.dram_tensor("w_out", [CIN, DIM], F32R, kind="ExternalInput").ap()
    outT_d = nc.dram_tensor("outT", [DIM, N], F32, kind="ExternalOutput").ap()

    with tile.TileContext(nc) as tc, ExitStack() as ctx:
        ppool = ctx.enter_context(tc.tile_pool(name="persist", bufs=1))

        # Persistent projection outputs (live across both scopes).
        z1T = [ppool.tile([128, N], F32R, tag=f"z1T{m}", name=f"z1T{m}")
               for m in range(2)]
        yhT = [ppool.tile([128, N], F32R, tag=f"yhT{m}", name=f"yhT{m}")
               for m in range(2)]
        xh_aug = [ppool.tile([128, HG * (DH + 1)], F32R, tag=f"xa{i}",
                             name=f"xa{i}") for i in range(NCH)]
        secm_sb = [ppool.tile([128, DH], F32R, tag=f"cm{p}", name=f"cm{p}")
                   for p in range(2)]
        rs = [ppool.tile([64, 1], F32, tag=f"rs{h}", name=f"rs{h}")
              for h in range(HG)]
        rcm = [ppool.tile([64, 1], F32, tag=f"rcm{h}", name=f"rcm{h}")
               for h in range(HG)]

        # ============ Scope 1: all projections + channel-attn logits ======
        with tc.tile_pool(name="proj_in", bufs=1) as ipool, \
             tc.tile_pool(name="psp", bufs=4, space="PSUM") as psp, \
             tc.tile_pool(name="pscm", bufs=1, space="PSUM") as pscm:
            # weights first (small), then x, z, y in consumption order
            wse1_t = [ipool.tile([128, CIN], BF16, tag=f"wse1_{k}",
                                 name=f"wse1_{k}") for k in range(KC)]
            wsa1_t = [ipool.tile([128, CIN], BF16, tag=f"wsa1_{k}",
                                 name=f"wsa1_{k}") for k in range(KC)]
            wse2_t = [ipool.tile([128, CIN], BF16, tag=f"wse2_{k}",
                                 name=f"wse2_{k}") for k in range(KC)]
            wsa2_t = [ipool.tile([128, CIN], BF16, tag=f"wsa2_{k}",
                                 name=f"wsa2_{k}") for k in range(KC)]
            for k in range(KC):
                nc.sync.dma_start(wse1_t[k][:], wse1_d[k * 128:(k + 1) * 128, :])
                nc.sync.dma_start(wsa1_t[k][:], wsa1_d[k * 128:(k + 1) * 128, :])
                nc.sync.dma_start(wse2_t[k][:], wse2_d[k * 128:(k + 1) * 128, :])
                nc.sync.dma_start(wsa2_t[k][:], wsa2_d[k * 128:(k + 1) * 128, :])
            xTt = [ipool.tile([128, N], BF16, tag=f"x{k}", name=f"x{k}")
                   for k in range(KC)]
            zTt = [ipool.tile([128, N], BF16, tag=f"z{k}", name=f"z{k}")
                   for k in range(KC)]
            yTt = [ipool.tile([128, N], BF16, tag=f"y{k}", name=f"y{k}")
                   for k in range(KC)]
            for k in range(KC):
                nc.sync.dma_start(xTt[k][:], xT_d[k * 128:(k + 1) * 128, :])
            for k in range(KC):
                nc.sync.dma_start(zTt[k][:], zT_d[k * 128:(k + 1) * 128, :])
            for k in range(KC):
                nc.sync.dma_start(yTt[k][:], yT_d[k * 128:(k + 1) * 128, :])

            cmps = [pscm.tile([64, DH], F32, tag=f"cmp{h}", name=f"cmp{h}")
                    for h in range(HG)]

            # --- xh (natural, augmented with ones) ---
            for i in range(NCH):
                ps = psp.tile([128, 512], F32, tag="pj", name=f"psx{i}")
                for k in range(KC):
                    nc.tensor.matmul(
                        ps[:, 0:CIN],
                        lhsT=xTt[k][:, i * 128:(i + 1) * 128],
                        rhs=wse1_t[k][:],
                        start=(k == 0), stop=(k == KC - 1),
                    )
                src = ps[:, 0:CIN].rearrange("p (h c) -> p h c", c=DH)
                dst = xh_aug[i][:].rearrange("p (h c) -> p h c", c=DH + 1)
                nc.vector.tensor_copy(dst[:, :, 0:DH], src)
                nc.scalar.activation(dst[:, :, DH:DH + 1], src[:, :, 0:1],
                                     mybir.ActivationFunctionType.Copy,
                                     bias=1.0, scale=0.0)

            # --- z1T (transposed projection) ---
            for m in range(2):
                for nb in range(4):
                    ps = psp.tile([128, 512], F32, tag="pj", name=f"psz{m}{nb}")
                    for k in range(KC):
                        nc.tensor.matmul(
                            ps[:],
                            lhsT=wsa1_t[k][:, m * 128:(m + 1) * 128],
                            rhs=zTt[k][:, nb * 512:(nb + 1) * 512],
                            start=(k == 0), stop=(k == KC - 1),
                        )
                    nc.any.tensor_copy(z1T[m][:, nb * 512:(nb + 1) * 512], ps[:])

            # --- z2 (natural, streamed) + channel-attn logits ---
            for i in range(NCH):
                ps2 = psp.tile([128, 512], F32, tag="pj", name=f"psz2_{i}")
                for k in range(KC):
                    nc.tensor.matmul(
                        ps2[:, 0:CIN],
                        lhsT=zTt[k][:, i * 128:(i + 1) * 128],
                        rhs=wse2_t[k][:],
                        start=(k == 0), stop=(k == KC - 1),
                    )
                z2n = ipool.tile([128, CIN], F32R, tag="z2n", bufs=3,
                                 name=f"z2n{i}")
                nc.any.tensor_copy(z2n[:], ps2[:, 0:CIN])
                for h in range(HG):
                    nc.tensor.matmul(
                        cmps[h][:],
                        lhsT=xh_aug[i][:, 65 * h:65 * h + DH],
                        rhs=z2n[:, DH * h:DH * (h + 1)],
                        start=(i == 0), stop=(i == NCH - 1),
                    )

            # --- yhT (transposed projection) ---
            for m in range(2):
                for nb in range(4):
                    ps = psp.tile([128, 512], F32, tag="pj", name=f"psy{m}{nb}")
                    for k in range(KC):
                        nc.tensor.matmul(
                            ps[:],
                            lhsT=wsa2_t[k][:, m * 128:(m + 1) * 128],
                            rhs=yTt[k][:, nb * 512:(nb + 1) * 512],
                            start=(k == 0), stop=(k == KC - 1),
                        )
                    nc.any.tensor_copy(yhT[m][:, nb * 512:(nb + 1) * 512], ps[:])

            # --- channel-attn softmax, DMA'd into pair-packed secm_sb ---
            for h in range(HG):
                p_, off = h // 2, 64 * (h % 2)
                st = ipool.tile([64, DH], F32R, tag="cmstage", bufs=4,
                                name=f"cmstage{h}")
                nc.scalar.activation(st[:], cmps[h][:], EXP,
                                     scale=CM_SCALE,
                                     accum_out=rs[h][0:64, 0:1])
                nc.vector.reciprocal(rcm[h][0:64, 0:1], rs[h][0:64, 0:1])
                nc.vector.tensor_scalar_mul(st[:], st[:], rcm[h][0:64, 0:1])
                nc.sync.dma_start(secm_sb[p_][off:off + 64, :], st[:])

        # ============ Scope 2: out2, spatial attention, final projection ==
        with tc.tile_pool(name="spat", bufs=1) as spool, \
             tc.tile_pool(name="pt", bufs=4) as ptpool, \
             tc.tile_pool(name="tails", bufs=3) as tpool, \
             tc.tile_pool(name="oout", bufs=4) as opool, \
             tc.tile_pool(name="psS", bufs=2, space="PSUM") as psS, \
             tc.tile_pool(name="psAV", bufs=4, space="PSUM") as psAV:

            # w_out as four 64-row slices (base partition 0) matching cat4
            wq = [spool.tile([64, DIM], F32R, tag=f"wq{q}", name=f"wq{q}")
                  for q in range(HG)]
            for q in range(HG):
                nc.sync.dma_start(wq[q][:], wout_d[q * 64:(q + 1) * 64, :])
            # cat^T staging: one [64, N] tile per head (this core's inner
            # channels [64h, 64h+64)); final projection contracts them with
            # matching 64-row slices of w_out
            cat4 = [spool.tile([64, N], F32R, tag=f"cat{h}", name=f"cat{h}")
                    for h in range(HG)]

            # --- out2^T = secm^T @ yh^T into cat staging ---
            for h in range(HG):
                p_, off = h // 2, 64 * (h % 2)
                for nb in range(4):
                    pso = psS.tile([128, 512], F32, tag="S", name=f"pso{h}{nb}")
                    nc.tensor.matmul(
                        pso[0:64, :],
                        lhsT=secm_sb[p_][off:off + 64, :],
                        rhs=yhT[p_][off:off + 64, nb * 512:(nb + 1) * 512],
                        start=True, stop=True,
                    )
                    nc.any.tensor_copy(cat4[h][:, nb * 512:(nb + 1) * 512],
                                       pso[0:64, :])

            # --- spatial attention (ib outer so final MMs can interleave) ---
            for ib in range(2):      # query mega-block of 1024
                for p_ in range(2):  # head pair
                    avs = [psAV.tile([128, 512], F32, tag="av",
                                     name=f"av{p_}{ib}{q}") for q in range(4)]
                    for j in range(NCH):  # key chunks
                        for hh in range(2):
                            h = 2 * p_ + hh
                            off = 64 * hh
                            spt = psS.tile([128, 1024], F32, tag="S",
                                           name=f"S{p_}{ib}{j}{hh}")
                            for s in range(2):
                                icol = ib * 1024 + s * 512
                                nc.tensor.matmul(
                                    spt[:, s * 512:(s + 1) * 512],
                                    lhsT=yhT[p_][off:off + 64,
                                                 j * 128:(j + 1) * 128],
                                    rhs=z1T[p_][off:off + 64, icol:icol + 512],
                                    start=True, stop=True,
                                )
                            ptt = ptpool.tile([128, 1024], F32R, tag="pt",
                                              name=f"pt{p_}{ib}{j}{hh}")
                            nc.scalar.activation(ptt[:], spt[:], EXP,
                                                 scale=SCALE)
                            for s in range(2):
                                nc.tensor.matmul(
                                    avs[2 * hh + s][0:DH + 1, :],
                                    lhsT=xh_aug[j][:, 65 * h:65 * h + DH + 1],
                                    rhs=ptt[:, s * 512:(s + 1) * 512],
                                    start=(j == 0), stop=(j == NCH - 1),
                                )
                    # tails: normalize out1 and add into cat staging
                    for hh in range(2):
                        h = 2 * p_ + hh
                        for s in range(2):
                            av = avs[2 * hh + s]
                            icol = ib * 1024 + s * 512
                            rc = tpool.tile([1, 512], F32, tag="rc",
                                            name=f"rc{p_}{ib}{hh}{s}")
                            nc.vector.reciprocal(rc[:], av[DH:DH + 1, :])
                            bc = tpool.tile([64, 512], F32, tag="bc",
                                            name=f"bc{p_}{ib}{hh}{s}")
                            nc.gpsimd.partition_broadcast(bc[:], rc[:])
                            tmp = tpool.tile([64, 512], F32, tag="tmp",
                                             name=f"tmp{p_}{ib}{hh}{s}")
                            nc.vector.tensor_mul(tmp[:], av[0:DH, :], bc[:])
                            dst = cat4[h][:, icol:icol + 512]
                            nc.vector.tensor_add(dst, tmp[:], dst)

                # final projection for the two query blocks finished above
                # (fills PE gaps while the next ib's exps run on ScalarE)
                for nb in (2 * ib, 2 * ib + 1):
                    for d in range(8):
                        psf = psS.tile([128, 512], F32, tag="S",
                                       name=f"psf{d}{nb}")
                        for q in range(HG):
                            nc.tensor.matmul(
                                psf[:],
                                lhsT=wq[q][:, d * 128:(d + 1) * 128],
                                rhs=cat4[q][:, nb * 512:(nb + 1) * 512],
                                start=(q == 0), stop=(q == HG - 1),
                            )
                        ob = opool.tile([128, 512], F32, tag="ob",
                                        name=f"ob{d}{nb}")
                        nc.any.tensor_copy(ob[:], psf[:])
                        nc.sync.dma_start(
                            outT_d[d * 128:(d + 1) * 128,
                                   nb * 512:(nb + 1) * 512],
                            ob[:],
                        )

    nc.compile()
    return nc


_NC_CACHE = {}


def _get_program():
    if "nc" not in _NC_CACHE:
        _NC_CACHE["nc"] = _build_program()
    return _NC_CACHE["nc"]


def _prep_input_maps(x, y, z, w_sa1, w_sa2, w_se1, w_se2, w_out):
    f32 = lambda a: np.ascontiguousarray(np.asarray(a, dtype=np.float32))
    bf16 = lambda a: np.ascontiguousarray(
        np.asarray(a, dtype=np.float32).astype(ml_dtypes.bfloat16))
    maps = []
    for c in range(NCORES):
        b, g = divmod(c, G)
        sl = slice(g * CIN, (g + 1) * CIN)
        maps.append({
            "xT": bf16(np.asarray(x)[b].T),
            "yT": bf16(np.asarray(y)[b].T),
            "zT": bf16(np.asarray(z)[b].T),
            "w_sa1": bf16(np.asarray(w_sa1)[:, sl]),
            "w_sa2": bf16(np.asarray(w_sa2)[:, sl]),
            "w_se1": bf16(np.asarray(w_se1)[:, sl]),
            "w_se2": bf16(np.asarray(w_se2)[:, sl]),
            "w_out": f32(np.asarray(w_out)[sl, :]),
        })
    return maps


def run(inputs, trace=False, trace_kwargs=None):
    """Run on hardware; returns (full_output, BassKernelResults)."""
    nc = _get_program()
    in_maps = _prep_input_maps(
        inputs["x"], inputs["y"], inputs["z"],
        inputs["w_sa1"], inputs["w_sa2"], inputs["w_se1"], inputs["w_se2"],
        inputs["w_out"],
    )
    res = run_bass_kernel_spmd(
        nc, in_maps, list(range(NCORES)), trace=trace,
        trace_kwargs=trace_kwargs or {},
    )
    out = np.zeros((B, N, DIM), dtype=np.float32)
    for c in range(NCORES):
        b, _g = divmod(c, G)
        out[b] += res.results[c]["outT"].T
    out += np.asarray(inputs["b_out"], dtype=np.float32)
    return out, res


def kernel(**inputs) -> np.ndarray:
    out, _ = run(inputs, trace=False)
    return out


# revision 11
# speedup vs baseline: 1.2597x; 1.1598x over previous
"""Trainium2 Bass kernel for nn_Attention_81037442941065.

Dual-attention module (spatial [b,h,n,n] + channel [b,h,d,d]) with
B=2, N=2048, DIM=1024, 16 heads of d=64.

Sharding: 8 cores = (2 batches) x (4 head-groups of 4 heads).
Each core computes its batch/head-group slice end-to-end and produces a
partial (over head groups) output projection; the host sums the 4 group
partials per batch (the "all-reduce after to_out") and adds b_out.

Dtypes: inputs x/y/z and the four projection weights are cast to bf16 on
the host (halves DMA + SBUF so all inputs stay resident; enables fast
weight load). Everything downstream of the projections runs in float32r
(1 cycle/row on the PE like bf16, ~11-bit mantissa) with fp32 PSUM.

Per-core layouts (everything "T" is [channels, tokens]):
  z1T, yhT   : [256, 2048]  (transposed projections, head h at rows 64*(h%2)
                             of tile h//2)
  xh_aug     : 16 tiles [128, 260] (natural layout per 128-token chunk;
               per head 65 cols = 64 channels + a ones column so the AV
               matmul also produces the softmax denominators)
  spatial    : S^T = yh @ z1^T computed [keys, queries]; exp on ScalarE
               (scale 1/8 fused, no max subtraction - logits are small);
               AV matmul lhsT=[xh|1] accumulates over key chunks -> rows
               0..63 = unnormalized out1^T, row 64 = sum of exp.
  channel    : [64,64] per head, one PSUM bank each; softmax via
               Exp+accum_out and per-partition reciprocal multiply.
"""

import sys

for _p in ("/opt/trn_rl_repo", "/opt/pypackages"):
    if _p not in sys.path:
        sys.path.insert(0, _p)

import ml_dtypes
import numpy as np
from contextlib import ExitStack

import concourse.bacc as bacc
import concourse.mybir as mybir
import concourse.tile as tile
from concourse.bass_utils import run_bass_kernel_spmd

F32 = mybir.dt.float32
F32R = mybir.dt.float32r
BF16 = mybir.dt.bfloat16
ATT = mybir.dt.bfloat16   # attention-internal matmul dtype
EXP = mybir.ActivationFunctionType.Exp

B, N, DIM = 2, 2048, 1024
HEADS, DH = 16, 64
G = 4              # head groups == cores per batch
HG = HEADS // G    # heads per group (4)
CIN = HG * DH      # inner channels per core (256)
NCORES = 8
KC = DIM // 128    # contraction chunks for projections (8)
NCH = N // 128     # 128-token chunks (16)
SCALE = DH ** -0.5            # 1/8
CM_SCALE = SCALE / (N / DH)   # 1/256


def _build_program():
    nc = bacc.Bacc(
        "TRN2", target_bir_lowering=False, debug=False, num_devices=NCORES
    )

    # ---- DRAM I/O ----
    xT_d = nc.dram_tensor("xT", [DIM, N], BF16, kind="ExternalInput").ap()
    yT_d = nc.dram_tensor("yT", [DIM, N], BF16, kind="ExternalInput").ap()
    zT_d = nc.dram_tensor("zT", [DIM, N], BF16, kind="ExternalInput").ap()
    wsa1_d = nc.dram_tensor("w_sa1", [DIM, CIN], BF16, kind="ExternalInput").ap()
    wsa2_d = nc.dram_tensor("w_sa2", [DIM, CIN], BF16, kind="ExternalInput").ap()
    wse1_d = nc.dram_tensor("w_se1", [DIM, CIN], BF16, kind="ExternalInput").ap()
    wse2_d = nc.dram_tensor("w_se2", [DIM, CIN], BF16, kind="ExternalInput").ap()
    wout_d = nc.dram_tensor("w_out", [CIN, DIM], ATT, kind="ExternalInput").ap()
    outT_d = nc.dram_tensor("outT", [DIM, N], F32, kind="ExternalOutput").ap()

    with tile.TileContext(nc) as tc, ExitStack() as ctx:
        ppool = ctx.enter_context(tc.tile_pool(name="persist", bufs=1))

        # Persistent projection outputs (live across both scopes).
        z1T = [ppool.tile([128, N], ATT, tag=f"z1T{m}", name=f"z1T{m}")
               for m in range(2)]
        yhT = [ppool.tile([128, N], ATT, tag=f"yhT{m}", name=f"yhT{m}")
               for m in range(2)]
        xh_aug = [ppool.tile([128, HG * (DH + 1)], ATT, tag=f"xa{i}",
                             name=f"xa{i}") for i in range(NCH)]
        secm_sb = [ppool.tile([128, DH], ATT, tag=f"cm{p}", name=f"cm{p}")
                   for p in range(2)]
        rs = [ppool.tile([64, 1], F32, tag=f"rs{h}", name=f"rs{h}")
              for h in range(HG)]
        rcm = [ppool.tile([64, 1], F32, tag=f"rcm{h}", name=f"rcm{h}")
               for h in range(HG)]

        # ============ Scope 1: all projections + channel-attn logits ======
        with tc.tile_pool(name="proj_in", bufs=1) as ipool, \
             tc.tile_pool(name="psp", bufs=4, space="PSUM") as psp, \
             tc.tile_pool(name="pscm", bufs=1, space="PSUM") as pscm:
            # weights first (small), then x, z, y in consumption order
            wse1_t = [ipool.tile([128, CIN], BF16, tag=f"wse1_{k}",
                                 name=f"wse1_{k}") for k in range(KC)]
            wsa1_t = [ipool.tile([128, CIN], BF16, tag=f"wsa1_{k}",
                                 name=f"wsa1_{k}") for k in range(KC)]
            wse2_t = [ipool.tile([128, CIN], BF16, tag=f"wse2_{k}",
                                 name=f"wse2_{k}") for k in range(KC)]
            wsa2_t = [ipool.tile([128, CIN], BF16, tag=f"wsa2_{k}",
                                 name=f"wsa2_{k}") for k in range(KC)]
            for k in range(KC):
                nc.sync.dma_start(wse1_t[k][:], wse1_d[k * 128:(k + 1) * 128, :])
                nc.sync.dma_start(wsa1_t[k][:], wsa1_d[k * 128:(k + 1) * 128, :])
                nc.sync.dma_start(wse2_t[k][:], wse2_d[k * 128:(k + 1) * 128, :])
                nc.sync.dma_start(wsa2_t[k][:], wsa2_d[k * 128:(k + 1) * 128, :])
            xTt = [ipool.tile([128, N], BF16, tag=f"x{k}", name=f"x{k}")
                   for k in range(KC)]
            zTt = [ipool.tile([128, N], BF16, tag=f"z{k}", name=f"z{k}")
                   for k in range(KC)]
            yTt = [ipool.tile([128, N], BF16, tag=f"y{k}", name=f"y{k}")
                   for k in range(KC)]
            for k in range(KC):
                nc.sync.dma_start(xTt[k][:], xT_d[k * 128:(k + 1) * 128, :])
            for k in range(KC):
                nc.sync.dma_start(zTt[k][:], zT_d[k * 128:(k + 1) * 128, :])
            for k in range(KC):
                nc.sync.dma_start(yTt[k][:], yT_d[k * 128:(k + 1) * 128, :])

            cmps = [pscm.tile([64, DH], F32, tag=f"cmp{h}", name=f"cmp{h}")
                    for h in range(HG)]

            # --- xh (natural, augmented with ones) ---
            for i in range(NCH):
                ps = psp.tile([128, 512], F32, tag="pj", name=f"psx{i}")
                for k in range(KC):
                    nc.tensor.matmul(
                        ps[:, 0:CIN],
                        lhsT=xTt[k][:, i * 128:(i + 1) * 128],
                        rhs=wse1_t[k][:],
                        start=(k == 0), stop=(k == KC - 1),
                    )
                src = ps[:, 0:CIN].rearrange("p (h c) -> p h c", c=DH)
                dst = xh_aug[i][:].rearrange("p (h c) -> p h c", c=DH + 1)
                nc.vector.tensor_copy(dst[:, :, 0:DH], src)
                nc.scalar.activation(dst[:, :, DH:DH + 1], src[:, :, 0:1],
                                     mybir.ActivationFunctionType.Copy,
                                     bias=1.0, scale=0.0)

            # --- z1T (transposed projection) ---
            for m in range(2):
                for nb in range(4):
                    ps = psp.tile([128, 512], F32, tag="pj", name=f"psz{m}{nb}")
                    for k in range(KC):
                        nc.tensor.matmul(
                            ps[:],
                            lhsT=wsa1_t[k][:, m * 128:(m + 1) * 128],
                            rhs=zTt[k][:, nb * 512:(nb + 1) * 512],
                            start=(k == 0), stop=(k == KC - 1),
                        )
                    nc.any.tensor_copy(z1T[m][:, nb * 512:(nb + 1) * 512], ps[:])

            # --- z2 (natural, streamed) + channel-attn logits ---
            for i in range(NCH):
                ps2 = psp.tile([128, 512], F32, tag="pj", name=f"psz2_{i}")
                for k in range(KC):
                    nc.tensor.matmul(
                        ps2[:, 0:CIN],
                        lhsT=zTt[k][:, i * 128:(i + 1) * 128],
                        rhs=wse2_t[k][:],
                        start=(k == 0), stop=(k == KC - 1),
                    )
                z2n = ipool.tile([128, CIN], ATT, tag="z2n", bufs=3,
                                 name=f"z2n{i}")
                nc.any.tensor_copy(z2n[:], ps2[:, 0:CIN])
                for h in range(HG):
                    nc.tensor.matmul(
                        cmps[h][:],
                        lhsT=xh_aug[i][:, 65 * h:65 * h + DH],
                        rhs=z2n[:, DH * h:DH * (h + 1)],
                        start=(i == 0), stop=(i == NCH - 1),
                    )

            # --- yhT (transposed projection) ---
            for m in range(2):
                for nb in range(4):
                    ps = psp.tile([128, 512], F32, tag="pj", name=f"psy{m}{nb}")
                    for k in range(KC):
                        nc.tensor.matmul(
                            ps[:],
                            lhsT=wsa2_t[k][:, m * 128:(m + 1) * 128],
                            rhs=yTt[k][:, nb * 512:(nb + 1) * 512],
                            start=(k == 0), stop=(k == KC - 1),
                        )
                    nc.any.tensor_copy(yhT[m][:, nb * 512:(nb + 1) * 512], ps[:])

            # --- channel-attn softmax, DMA'd into pair-packed secm_sb ---
            for h in range(HG):
                p_, off = h // 2, 64 * (h % 2)
                st = ipool.tile([64, DH], ATT, tag="cmstage", bufs=4,
                                name=f"cmstage{h}")
                nc.scalar.activation(st[:], cmps[h][:], EXP,
                                     scale=CM_SCALE,
                                     accum_out=rs[h][0:64, 0:1])
                nc.vector.reciprocal(rcm[h][0:64, 0:1], rs[h][0:64, 0:1])
                nc.vector.tensor_scalar_mul(st[:], st[:], rcm[h][0:64, 0:1])
                nc.sync.dma_start(secm_sb[p_][off:off + 64, :], st[:])

        # ============ Scope 2: out2, spatial attention, final projection ==
        with tc.tile_pool(name="spat", bufs=1) as spool, \
             tc.tile_pool(name="pt", bufs=4) as ptpool, \
             tc.tile_pool(name="tails", bufs=3) as tpool, \
             tc.tile_pool(name="oout", bufs=4) as opool, \
             tc.tile_pool(name="psS", bufs=2, space="PSUM") as psS, \
             tc.tile_pool(name="psAV", bufs=4, space="PSUM") as psAV:

            # w_out as four 64-row slices (base partition 0) matching cat4
            wq = [spool.tile([64, DIM], ATT, tag=f"wq{q}", name=f"wq{q}")
                  for q in range(HG)]
            for q in range(HG):
                nc.sync.dma_start(wq[q][:], wout_d[q * 64:(q + 1) * 64, :])
            # cat^T staging: one [64, N] tile per head (this core's inner
            # channels [64h, 64h+64)); final projection contracts them with
            # matching 64-row slices of w_out
            cat4 = [spool.tile([64, N], ATT, tag=f"cat{h}", name=f"cat{h}")
                    for h in range(HG)]

            # --- out2^T = secm^T @ yh^T into cat staging ---
            for h in range(HG):
                p_, off = h // 2, 64 * (h % 2)
                for nb in range(4):
                    pso = psS.tile([128, 512], F32, tag="S", name=f"pso{h}{nb}")
                    nc.tensor.matmul(
                        pso[0:64, :],
                        lhsT=secm_sb[p_][off:off + 64, :],
                        rhs=yhT[p_][off:off + 64, nb * 512:(nb + 1) * 512],
                        start=True, stop=True,
                    )
                    nc.any.tensor_copy(cat4[h][:, nb * 512:(nb + 1) * 512],
                                       pso[0:64, :])

            # --- spatial attention (ib outer so final MMs can interleave) ---
            for ib in range(2):      # query mega-block of 1024
                for p_ in range(2):  # head pair
                    avs = [psAV.tile([128, 512], F32, tag="av",
                                     name=f"av{p_}{ib}{q}") for q in range(4)]
                    for j in range(NCH):  # key chunks
                        for hh in range(2):
                            h = 2 * p_ + hh
                            off = 64 * hh
                            spt = psS.tile([128, 1024], F32, tag="S",
                                           name=f"S{p_}{ib}{j}{hh}")
                            for s in range(2):
                                icol = ib * 1024 + s * 512
                                nc.tensor.matmul(
                                    spt[:, s * 512:(s + 1) * 512],
                                    lhsT=yhT[p_][off:off + 64,
                                                 j * 128:(j + 1) * 128],
                                    rhs=z1T[p_][off:off + 64, icol:icol + 512],
                                    start=True, stop=True,
                                )
                            ptt = ptpool.tile([128, 1024], ATT, tag="pt",
                                              name=f"pt{p_}{ib}{j}{hh}")
                            nc.scalar.activation(ptt[:], spt[:], EXP,
                                                 scale=SCALE)
                            for s in range(2):
                                nc.tensor.matmul(
                                    avs[2 * hh + s][0:DH + 1, :],
                                    lhsT=xh_aug[j][:, 65 * h:65 * h + DH + 1],
                                    rhs=ptt[:, s * 512:(s + 1) * 512],
                                    start=(j == 0), stop=(j == NCH - 1),
                                )
                    # tails: normalize out1 and add into cat staging
                    for hh in range(2):
                        h = 2 * p_ + hh
                        for s in range(2):
                            av = avs[2 * hh + s]
                            icol = ib * 1024 + s * 512
                            rc = tpool.tile([1, 512], F32, tag="rc",
                                            name=f"rc{p_}{ib}{hh}{s}")
                            nc.vector.reciprocal(rc[:], av[DH:DH + 1, :])
                            bc = tpool.tile([64, 512], F32, tag="bc",
                                            name=f"bc{p_}{ib}{hh}{s}")
                            nc.gpsimd.partition_broadcast(bc[:], rc[:])
                            tmp = tpool.tile([64, 512], F32, tag="tmp",
                                             name=f"tmp{p_}{ib}{hh}{s}")
                            nc.vector.tensor_mul(tmp[:], av[0:DH, :], bc[:])
                            dst = cat4[h][:, icol:icol + 512]
                            nc.vector.tensor_add(dst, tmp[:], dst)

                # final projection for the two query blocks finished above
                # (fills PE gaps while the next ib's exps run on ScalarE)
                for nb in (2 * ib, 2 * ib + 1):
                    for d in range(8):
                        psf = psS.tile([128, 512], F32, tag="S",
                                       name=f"psf{d}{nb}")
                        for q in range(HG):
                            nc.tensor.matmul(
                                psf[:],
                                lhsT=wq[q][:, d * 128:(d + 1) * 128],
                                rhs=cat4[q][:, nb * 512:(nb + 1) * 512],
                                start=(q == 0), stop=(q == HG - 1),
                            )
                        ob = opool.tile([128, 512], F32, tag="ob",
                                        name=f"ob{d}{nb}")
                        nc.any.tensor_copy(ob[:], psf[:])
                        nc.sync.dma_start(
                            outT_d[d * 128:(d + 1) * 128,
                                   nb * 512:(nb + 1) * 512],
                            ob[:],
                        )

    nc.compile()
    return nc


_NC_CACHE = {}


def _get_program():
    if "nc" not in _NC_CACHE:
        _NC_CACHE["nc"] = _build_program()
    return _NC_CACHE["nc"]


def _prep_input_maps(x, y, z, w_sa1, w_sa2, w_se1, w_se2, w_out):
    f32 = lambda a: np.ascontiguousarray(np.asarray(a, dtype=np.float32))
    bf16 = lambda a: np.ascontiguousarray(
        np.asarray(a, dtype=np.float32).astype(ml_dtypes.bfloat16))
    maps = []
    for c in range(NCORES):
        b, g = divmod(c, G)
        sl = slice(g * CIN, (g + 1) * CIN)
        maps.append({
            "xT": bf16(np.asarray(x)[b].T),
            "yT": bf16(np.asarray(y)[b].T),
            "zT": bf16(np.asarray(z)[b].T),
            "w_sa1": bf16(np.asarray(w_sa1)[:, sl]),
            "w_sa2": bf16(np.asarray(w_sa2)[:, sl]),
            "w_se1": bf16(np.asarray(w_se1)[:, sl]),
            "w_se2": bf16(np.asarray(w_se2)[:, sl]),
            "w_out": bf16(np.asarray(w_out)[sl, :]),
        })
    return maps


def run(inputs, trace=False, trace_kwargs=None):
    """Run on hardware; returns (full_output, BassKernelResults)."""
    nc = _get_program()
    in_maps = _prep_input_maps(
        inputs["x"], inputs["y"], inputs["z"],
        inputs["w_sa1"], inputs["w_sa2"], inputs["w_se1"], inputs["w_se2"],
        inputs["w_out"],
    )
    res = run_bass_kernel_spmd(
        nc, in_maps, list(range(NCORES)), trace=trace,
        trace_kwargs=trace_kwargs or {},
    )
    out = np.zeros((B, N, DIM), dtype=np.float32)
    for c in range(NCORES):
        b, _g = divmod(c, G)
        out[b] += res.results[c]["outT"].T
    out += np.asarray(inputs["b_out"], dtype=np.float32)
    return out, res


def kernel(**inputs) -> np.ndarray:
    out, _ = run(inputs, trace=False)
    return out
